# revision 40
# baseline (speedup 1.0000x reference)
"""Multi-head self-attention Trainium2 kernel (8-core SPMD, full IO).

Problem: x:(2,2048,1024) f32; Wq/Wk/Wv/Wo:(1024,1024); bo:(1024,)
  out = softmax((xWq)(xWk)^T / 8) (xWv) reshaped @ Wo + bo

Sharding: data parallel on batch N=2 x tensor parallel on 16 heads in
4 groups of 4 heads.  Core c handles batch c//4, heads [4*(c%4), 4*(c%4)+4).
Each core computes a partial fc_out product (2048,1024) in bf16; the host
sums the 4 head-group partials per batch (f32) and adds the bias.

v3 schedule (ACT-engine exp stream is the critical path; keep it fed):
  - K/V chunk 0 + Q chunk 0 projected, then attention starts immediately;
    K/V chunks 1-3 are emitted between the m-blocks that first need them.
  - qc-outer attention: per 512-token q-chunk and head pair hm, a
    software-pipelined m-loop: scores S^T (PE, row-grouped j pair), exp
    (ACT, one [128,1024] instr for both heads), A@V lagging two
    iterations so the PE never waits on ACT.
  - remaining Q projections and fc_out matmuls feed into PE slack, max 2
    per iteration; fc_out is gated until iter 6 of its block so it never
    stalls the queue on the previous block's O^T staging DMAs.
  - softmax denominator from a ones-column of V; normalize on-chip:
    scratch copy (frees psum fast), gpsimd partition_broadcast of the
    denominator row, reciprocal, fused multiply into bf16 O^T staging.
"""

import os

import numpy as np

import concourse.bass as bass
import concourse.tile as tile
from concourse import bacc, mybir
from concourse import bass_utils

F32 = mybir.dt.float32
BF16 = mybir.dt.bfloat16

EMBED = 1024
SEQ = 2048
NB = 2
HEADS = 16
D = 64
NCORES = 8
GROUPS = 4
HG = HEADS // GROUPS  # 4 heads per core
DG = HG * D  # 256 dims per core
KC = EMBED // 128  # 8 contraction chunks
TCH = 512  # token chunk
NT = SEQ // TCH  # 4 chunks

_MM_DTYPE_NAME = "bfloat16"
MD = BF16

LAST_RESULTS = None
_CACHED_NC = {}


def build_nc():
    nc = bacc.Bacc("TRN2", target_bir_lowering=False, debug=False,
                   num_devices=NCORES)

    # all inputs are pre-packed host-side to partition-major tiles so the
    # DMAs are fully contiguous per partition
    xT = nc.dram_tensor("xT", (NT, 128, KC * TCH), MD, kind="ExternalInput").ap()
    wq = nc.dram_tensor("wq", (128, KC * DG), MD, kind="ExternalInput").ap()
    wk = nc.dram_tensor("wk", (128, KC * DG), MD, kind="ExternalInput").ap()
    wv = nc.dram_tensor("wv", (128, KC * DG), MD, kind="ExternalInput").ap()
    wo = nc.dram_tensor("wo", (128, (DG // 128) * EMBED), MD,
                        kind="ExternalInput").ap()
    y = nc.dram_tensor("y", (SEQ, EMBED), MD, kind="ExternalOutput").ap()

    with tile.TileContext(nc) as tc:
        with (
            tc.tile_pool(name="weights", bufs=1) as wpool,
            tc.tile_pool(name="qk", bufs=1) as qkpool,
            tc.tile_pool(name="vpool", bufs=1) as vpool,
            tc.tile_pool(name="otpool", bufs=1) as otpool,
            tc.tile_pool(name="xchunk", bufs=3) as xpool,
            tc.tile_pool(name="epool", bufs=4) as epool,
            tc.tile_pool(name="vtpool", bufs=1) as vtpool,
            tc.tile_pool(name="vttpool", bufs=1) as vttpool,
            tc.tile_pool(name="scratch", bufs=1) as spool,
            tc.tile_pool(name="stage", bufs=1) as stpool,
            tc.tile_pool(name="rbc", bufs=1) as rbpool,
            tc.tile_pool(name="denr", bufs=1) as drpool,
            tc.tile_pool(name="ystage", bufs=2) as ypool,
            tc.tile_pool(name="psum", bufs=2, space="PSUM") as pspool,
            tc.tile_pool(name="psum_o", bufs=1, space="PSUM") as popool,
            tc.tile_pool(name="psum_fcq", bufs=1, space="PSUM") as fqpool,
        ):
            # ---- weights + x chunks ----
            wk_sb = wpool.tile([128, KC, DG], MD)
            wv_sb = wpool.tile([128, KC, DG], MD)
            wq_sb = wpool.tile([128, KC, DG], MD)
            wo_sb = wpool.tile([128, DG // 128, EMBED], MD)
            # spread the startup DMAs over two queues, splitting the
            # first-matmul inputs (wk + x chunk 0) in half across both so
            # the K projection can start as early as possible
            wkr = wk.rearrange("p (c n) -> p c n", c=KC)
            nc.sync.dma_start(out=wk_sb[:, 0:KC // 2], in_=wkr[:, 0:KC // 2])
            nc.scalar.dma_start(out=wk_sb[:, KC // 2:], in_=wkr[:, KC // 2:])

            xcs = []
            for t in range(NT):
                xc = xpool.tile([128, KC, TCH], MD, name=f"xc{t}", tag=f"xc{t}")
                xv = xT[t].rearrange("p (c s) -> p c s", c=KC)
                if t == 0:
                    nc.sync.dma_start(out=xc[:, 0:KC // 2], in_=xv[:, 0:KC // 2])
                    nc.scalar.dma_start(out=xc[:, KC // 2:], in_=xv[:, KC // 2:])
                    nc.scalar.dma_start(
                        out=wv_sb, in_=wv.rearrange("p (c n) -> p c n", c=KC))
                    nc.scalar.dma_start(
                        out=wq_sb, in_=wq.rearrange("p (c n) -> p c n", c=KC))
                else:
                    nc.sync.dma_start(out=xc, in_=xv)
                xcs.append(xc)
            nc.scalar.dma_start(
                out=wo_sb, in_=wo.rearrange("p (c n) -> p c n", c=DG // 128))

            QTs = [qkpool.tile([128, 2, TCH], MD, name=f"qt{t}", tag=f"qt{t}")
                   for t in range(NT)]
            KTs = [qkpool.tile([128, 2, TCH], MD, name=f"kt{t}", tag=f"kt{t}")
                   for t in range(NT)]
            Vs = [vpool.tile([128, 4, HG, D + 1], MD, name=f"v{t}", tag=f"v{t}")
                  for t in range(NT)]
            for t in range(NT):
                nc.vector.memset(Vs[t][:, :, :, D:D + 1], 1.0)

            # ---- emitters ----
            def kv_proj(t):
                # K^T and V^T via identical w-stationary 512-col chains;
                # V is then turned token-major by 8 xbar transpose DMAs
                xc = xcs[t]
                pk = pspool.tile([128, 2 * TCH], F32, name="ps", tag="ps")
                pv = pspool.tile([128, 2 * TCH], F32, name="ps", tag="ps")
                for kc in range(KC):
                    for mt in range(2):
                        nc.tensor.matmul(
                            pk[:, mt * TCH:(mt + 1) * TCH],
                            wk_sb[:, kc, mt * 128:(mt + 1) * 128],
                            xc[:, kc, :],
                            start=(kc == 0), stop=(kc == KC - 1))
                        nc.tensor.matmul(
                            pv[:, mt * TCH:(mt + 1) * TCH],
                            wv_sb[:, kc, mt * 128:(mt + 1) * 128],
                            xc[:, kc, :],
                            start=(kc == 0), stop=(kc == KC - 1))
                nc.vector.tensor_copy(
                    out=KTs[t], in_=pk.rearrange("p (m s) -> p m s", m=2))
                vt = vtpool.tile([128, 2, TCH], MD, name="vt", tag="vt")
                nc.vector.tensor_copy(
                    out=vt, in_=pv.rearrange("p (m s) -> p m s", m=2))
                vtt = vttpool.tile([128, 4, 2 * 128], MD, name="vtt", tag="vtt")
                for mt in range(2):
                    for blk in range(4):
                        nc.sync.dma_start_transpose(
                            out=vtt[:, blk, mt * 128:(mt + 1) * 128],
                            in_=vt[:, mt, blk * 128:(blk + 1) * 128])
                nc.vector.tensor_copy(
                    out=Vs[t][:, :, :, 0:D],
                    in_=vtt.rearrange("p b (h d) -> p b h d", h=HG))

            def q_proj_ops(t):
                """Feeder items ('q', op) for projecting Q chunk t."""
                ops = []
                pq_box = []

                def alloc():
                    pq_box.append(fqpool.tile([128, 2 * TCH], F32,
                                              name="fq", tag="fq"))
                ops.append(('q', alloc))
                for kc in range(KC):
                    for mt in range(2):
                        def mm(kc=kc, mt=mt):
                            nc.tensor.matmul(
                                pq_box[0][:, mt * TCH:(mt + 1) * TCH],
                                wq_sb[:, kc, mt * 128:(mt + 1) * 128],
                                xcs[t][:, kc, :],
                                start=(kc == 0), stop=(kc == KC - 1))
                        ops.append(('q', mm))

                def cp():
                    nc.vector.tensor_copy(
                        out=QTs[t],
                        in_=pq_box[0].rearrange("p (m s) -> p m s", m=2))
                ops.append(('q', cp))
                return ops

            def fc_ops(qc, pool=None, tag="fq"):
                """Feeder items ('fc', op) for fc_out of q-chunk qc."""
                ops = []
                for tt in range(TCH // 128):
                    tok = qc * TCH + tt * 128
                    pf_box = []

                    def alloc(pool=pool, tag=tag):
                        pf_box.append((pool or fqpool).tile(
                            [128, 1024], F32, name=tag, tag=tag))
                    ops.append(('fc', alloc))
                    for hm in range(2):
                        for nch in range(2):
                            def mm(hm=hm, nch=nch, tok=tok):
                                nc.tensor.matmul(
                                    pf_box[0][:, nch * 512:(nch + 1) * 512],
                                    OT2[:, hm, tok:tok + 128],
                                    wo_sb[:, hm, nch * 512:(nch + 1) * 512],
                                    start=(hm == 0), stop=(hm == 1))
                            ops.append(('fc', mm))

                    def fin(tok=tok):
                        ys = ypool.tile([128, 1024], MD, name="ys", tag="ys")
                        nc.vector.tensor_copy(out=ys, in_=pf_box[0])
                        nc.gpsimd.dma_start(out=y[tok:tok + 128, :], in_=ys)
                    ops.append(('fc', fin))
                return ops

            OT2 = otpool.tile([128, 2, SEQ], MD)

            # ---- prologue: first K/V chunk + first Q chunk ----
            kv_proj(0)
            for _, op in q_proj_ops(0):
                op()

            feeders = {
                0: q_proj_ops(1),
                1: q_proj_ops(2) + fc_ops(0),
                2: q_proj_ops(3) + fc_ops(1),
                3: fc_ops(2),
            }

            # ---- fused attention ----
            for qc in range(NT):
                qs = slice(qc * TCH, (qc + 1) * TCH)
                pending = feeders.get(qc, [])
                n_iters_left = 2 * 16
                it = 0

                for hm in range(2):
                    po = popool.tile([D + 1, 2 * TCH], F32, name="po", tag="po")
                    es = [None, None]  # lag-2 pipeline: es[m-2], es[m-1]
                    for m in range(16):
                        # late K/V chunks, right before first use (qc 0 only)
                        if qc == 0 and hm == 0 and m in (4, 8, 12):
                            kv_proj(m // 4)
                        ps = pspool.tile([128, 2 * TCH], F32,
                                         name="ps", tag="ps")
                        for j in range(2):
                            nc.tensor.matmul(
                                ps[:, j * TCH:(j + 1) * TCH],
                                KTs[m // 4][j * D:(j + 1) * D, hm,
                                            (m % 4) * 128:(m % 4 + 1) * 128],
                                QTs[qc][j * D:(j + 1) * D, hm, :],
                                start=True, stop=True)
                        e = epool.tile([128, 2 * TCH], MD, name="e", tag="e")
                        nc.scalar.activation(
                            out=e, in_=ps,
                            func=mybir.ActivationFunctionType.Exp,
                            scale=1.0 / np.sqrt(D))
                        if es[0] is not None:
                            for j in range(2):
                                nc.tensor.matmul(
                                    po[:, j * TCH:(j + 1) * TCH],
                                    Vs[(m - 2) // 4][:, (m - 2) % 4,
                                                     2 * hm + j, :],
                                    es[0][:, j * TCH:(j + 1) * TCH],
                                    start=(m == 2), stop=False)
                        es = [es[1], e]
                        # feed pending work into PE slack (<=2 per iter;
                        # fc gated to iter>=6 of this qc block)
                        n_pop = min(2, -(-len(pending) // n_iters_left)
                                    if n_iters_left > 0 else len(pending))
                        for _ in range(n_pop):
                            if not pending:
                                break
                            tag, op = pending[0]
                            if tag == 'fc' and it < 10:
                                break
                            pending.pop(0)
                            op()
                        n_iters_left -= 1
                        it += 1
                    # drain the two lagged AV pairs
                    for mm_, e_ in ((14, es[0]), (15, es[1])):
                        for j in range(2):
                            nc.tensor.matmul(
                                po[:, j * TCH:(j + 1) * TCH],
                                Vs[3][:, 3 if mm_ == 15 else 2,
                                      2 * hm + j, :],
                                e_[:, j * TCH:(j + 1) * TCH],
                                start=False, stop=(mm_ == 15))

                    # normalize: scratch copy frees psum, then
                    # broadcast(den row) -> reciprocal -> fused mul
                    dn = drpool.tile([1, 2 * TCH], F32, name="dn", tag="dn")
                    nc.vector.tensor_copy(out=dn, in_=po[D:D + 1, :])
                    sc = spool.tile([D + 1, 2 * TCH], F32, name="sc", tag="sc")
                    nc.vector.tensor_copy(out=sc, in_=po)
                    dr = drpool.tile([1, 2 * TCH], F32, name="dr", tag="dr")
                    nc.vector.reciprocal_approx_fast(out=dr, in_=dn)
                    rb = rbpool.tile([D, 2 * TCH], F32, name="rb", tag="rb")
                    nc.gpsimd.partition_broadcast(rb, dr)
                    st = stpool.tile([D, 2 * TCH], MD, name="st", tag="st")
                    nc.vector.tensor_mul(st, sc[0:D, :], rb)
                    for j in range(2):
                        nc.gpsimd.dma_start(
                            out=OT2[j * D:(j + 1) * D, hm, qs],
                            in_=st[:, j * TCH:(j + 1) * TCH])

                while pending:
                    pending.pop(0)[1]()

            # drain fc for the last q-chunk from the (now idle) attention
            # psum pool so consecutive groups pipeline instead of
            # serializing on a single buffer
            for _, op in fc_ops(3, pool=pspool, tag="ps"):
                op()

    nc.compile()
    return nc


def _pack_w(w):
    """(1024, DG) -> (128, KC*DG), row p holding chunks c of rows c*128+p."""
    return np.ascontiguousarray(
        w.reshape(KC, 128, DG).transpose(1, 0, 2).reshape(128, KC * DG))


def shard_inputs(x, Wv, Wk, Wq, Wo):
    import ml_dtypes
    wire = ml_dtypes.bfloat16
    in_maps = []
    for c in range(NCORES):
        n, g = divmod(c, GROUPS)
        cols = slice(g * DG, (g + 1) * DG)
        xt = np.asarray(x[n], np.float32).T  # (EMBED, SEQ)
        xp = xt.reshape(KC, 128, NT, TCH).transpose(2, 1, 0, 3) \
            .reshape(NT, 128, KC * TCH)
        wop = np.asarray(Wo, np.float32)[cols, :] \
            .reshape(DG // 128, 128, EMBED).transpose(1, 0, 2) \
            .reshape(128, (DG // 128) * EMBED)
        in_maps.append({
            "xT": np.ascontiguousarray(xp).astype(wire),
            "wq": _pack_w(np.asarray(Wq, np.float32)[:, cols]).astype(wire),
            "wk": _pack_w(np.asarray(Wk, np.float32)[:, cols]).astype(wire),
            "wv": _pack_w(np.asarray(Wv, np.float32)[:, cols]).astype(wire),
            "wo": np.ascontiguousarray(wop).astype(wire),
        })
    return in_maps


def kernel(x, Wv, Wk, Wq, Wo, bo):
    global LAST_RESULTS
    x = np.asarray(x, np.float32)
    in_maps = shard_inputs(x, Wv, Wk, Wq, Wo)

    if "nc" not in _CACHED_NC:
        _CACHED_NC["nc"] = build_nc()
    nc = _CACHED_NC["nc"]

    trace = os.environ.get("MHA_TRACE", "0") == "1"
    res = bass_utils.run_bass_kernel_spmd(
        nc, in_maps, core_ids=list(range(NCORES)), trace=trace)
    LAST_RESULTS = res

    bo = np.asarray(bo, np.float32)
    out = np.empty((NB, SEQ, EMBED), np.float32)
    for n in range(NB):
        acc = res.results[n * GROUPS]["y"].astype(np.float32)
        for g in range(1, GROUPS):
            acc = acc + res.results[n * GROUPS + g]["y"].astype(np.float32)
        out[n] = acc + bo[None, :]
    return out


# revision 42
# speedup vs baseline: 1.2317x; 1.2317x over previous
"""Multi-head self-attention Trainium2 kernel (8-core SPMD, full IO).

Problem: x:(2,2048,1024) f32; Wq/Wk/Wv/Wo:(1024,1024); bo:(1024,)
  out = softmax((xWq)(xWk)^T / 8) (xWv) reshaped @ Wo + bo

Sharding: data parallel on batch N=2 x tensor parallel on 16 heads in
4 groups of 4 heads.  Core c handles batch c//4, heads [4*(c%4), 4*(c%4)+4).
Each core computes a partial fc_out product (2048,1024) in bf16; the host
sums the 4 head-group partials per batch (f32) and adds the bias.

v3 schedule (ACT-engine exp stream is the critical path; keep it fed):
  - K/V chunk 0 + Q chunk 0 projected, then attention starts immediately;
    K/V chunks 1-3 are emitted between the m-blocks that first need them.
  - qc-outer attention: per 512-token q-chunk and head pair hm, a
    software-pipelined m-loop: scores S^T (PE, row-grouped j pair), exp
    (ACT, one [128,1024] instr for both heads), A@V lagging two
    iterations so the PE never waits on ACT.
  - remaining Q projections and fc_out matmuls feed into PE slack, max 2
    per iteration; fc_out is gated until iter 6 of its block so it never
    stalls the queue on the previous block's O^T staging DMAs.
  - softmax denominator from a ones-column of V; normalize on-chip:
    scratch copy (frees psum fast), gpsimd partition_broadcast of the
    denominator row, reciprocal, fused multiply into bf16 O^T staging.
"""

import os

import numpy as np

import concourse.bass as bass
import concourse.tile as tile
from concourse import bacc, mybir
from concourse import bass_utils

F32 = mybir.dt.float32
BF16 = mybir.dt.bfloat16

EMBED = 1024
SEQ = 2048
NB = 2
HEADS = 16
D = 64
NCORES = 8
GROUPS = 4
HG = HEADS // GROUPS  # 4 heads per core
DG = HG * D  # 256 dims per core
KC = EMBED // 128  # 8 contraction chunks
TCH = 512  # token chunk
NT = SEQ // TCH  # 4 chunks

_MM_DTYPE_NAME = "bfloat16"
MD = BF16

LAST_RESULTS = None
_CACHED_NC = {}


def build_nc():
    nc = bacc.Bacc("TRN2", target_bir_lowering=False, debug=False,
                   num_devices=NCORES)

    # all inputs are pre-packed host-side to partition-major tiles so the
    # DMAs are fully contiguous per partition
    xT = nc.dram_tensor("xT", (NT, 128, KC * TCH), MD, kind="ExternalInput").ap()
    wq = nc.dram_tensor("wq", (128, KC * DG), MD, kind="ExternalInput").ap()
    wk = nc.dram_tensor("wk", (128, KC * DG), MD, kind="ExternalInput").ap()
    wv = nc.dram_tensor("wv", (128, KC * DG), MD, kind="ExternalInput").ap()
    wo = nc.dram_tensor("wo", (128, (DG // 128) * EMBED), MD,
                        kind="ExternalInput").ap()
    y = nc.dram_tensor("y", (SEQ, EMBED), MD, kind="ExternalOutput").ap()

    with tile.TileContext(nc) as tc:
        with (
            tc.tile_pool(name="weights", bufs=1) as wpool,
            tc.tile_pool(name="qk", bufs=1) as qkpool,
            tc.tile_pool(name="vpool", bufs=1) as vpool,
            tc.tile_pool(name="otpool", bufs=1) as otpool,
            tc.tile_pool(name="xchunk", bufs=3) as xpool,
            tc.tile_pool(name="epool", bufs=4) as epool,

            tc.tile_pool(name="scratch", bufs=1) as spool,
            tc.tile_pool(name="stage", bufs=1) as stpool,
            tc.tile_pool(name="rbc", bufs=1) as rbpool,
            tc.tile_pool(name="denr", bufs=1) as drpool,
            tc.tile_pool(name="ystage", bufs=2) as ypool,
            tc.tile_pool(name="psum", bufs=2, space="PSUM") as pspool,
            tc.tile_pool(name="psum_o", bufs=1, space="PSUM") as popool,
            tc.tile_pool(name="psum_fcq", bufs=1, space="PSUM") as fqpool,
        ):
            # ---- weights + x chunks ----
            wk_sb = wpool.tile([128, KC, DG], MD)
            wv_sb = wpool.tile([128, KC, DG], MD)
            wq_sb = wpool.tile([128, KC, DG], MD)
            wo_sb = wpool.tile([128, DG // 128, EMBED], MD)
            # spread the startup DMAs over two queues, splitting the
            # first-matmul inputs (wk + x chunk 0) in half across both so
            # the K projection can start as early as possible
            wkr = wk.rearrange("p (c n) -> p c n", c=KC)
            nc.sync.dma_start(out=wk_sb[:, 0:KC // 2], in_=wkr[:, 0:KC // 2])
            nc.scalar.dma_start(out=wk_sb[:, KC // 2:], in_=wkr[:, KC // 2:])

            xcs = []
            for t in range(NT):
                xc = xpool.tile([128, KC, TCH], MD, name=f"xc{t}", tag=f"xc{t}")
                xv = xT[t].rearrange("p (c s) -> p c s", c=KC)
                if t == 0:
                    nc.sync.dma_start(out=xc[:, 0:KC // 2], in_=xv[:, 0:KC // 2])
                    nc.scalar.dma_start(out=xc[:, KC // 2:], in_=xv[:, KC // 2:])
                    nc.scalar.dma_start(
                        out=wv_sb, in_=wv.rearrange("p (c n) -> p c n", c=KC))
                    nc.scalar.dma_start(
                        out=wq_sb, in_=wq.rearrange("p (c n) -> p c n", c=KC))
                else:
                    nc.sync.dma_start(out=xc, in_=xv)
                xcs.append(xc)
            nc.scalar.dma_start(
                out=wo_sb, in_=wo.rearrange("p (c n) -> p c n", c=DG // 128))

            QTs = [qkpool.tile([128, 2, TCH], MD, name=f"qt{t}", tag=f"qt{t}")
                   for t in range(NT)]
            KTs = [qkpool.tile([128, 2, TCH], MD, name=f"kt{t}", tag=f"kt{t}")
                   for t in range(NT)]
            Vs = [vpool.tile([128, 4, HG, D + 1], MD, name=f"v{t}", tag=f"v{t}")
                  for t in range(NT)]
            for t in range(NT):
                nc.vector.memset(Vs[t][:, :, :, D:D + 1], 1.0)

            # ---- emitters ----
            def kv_proj(t):
                xc = xcs[t]
                pk = pspool.tile([128, 2 * TCH], F32, name="ps", tag="ps")
                for kc in range(KC):
                    for mt in range(2):
                        nc.tensor.matmul(
                            pk[:, mt * TCH:(mt + 1) * TCH],
                            wk_sb[:, kc, mt * 128:(mt + 1) * 128],
                            xc[:, kc, :],
                            start=(kc == 0), stop=(kc == KC - 1))
                nc.vector.tensor_copy(
                    out=KTs[t], in_=pk.rearrange("p (m s) -> p m s", m=2))
                for tp in range(2):
                    pv = pspool.tile([128, 2 * TCH], F32, name="ps", tag="ps")
                    for kc in range(KC):
                        for k in range(2):
                            ti = 2 * tp + k
                            nc.tensor.matmul(
                                pv[:, k * TCH:k * TCH + DG],
                                xc[:, kc, ti * 128:(ti + 1) * 128],
                                wv_sb[:, kc, :],
                                start=(kc == 0), stop=(kc == KC - 1))
                    for k in range(2):
                        nc.vector.tensor_copy(
                            out=Vs[t][:, 2 * tp + k, :, 0:D],
                            in_=pv[:, k * TCH:k * TCH + DG]
                            .rearrange("p (h d) -> p h d", h=HG))

            def q_proj_ops(t):
                """Feeder items ('q', op) for projecting Q chunk t."""
                ops = []
                pq_box = []

                def alloc():
                    pq_box.append(fqpool.tile([128, 2 * TCH], F32,
                                              name="fq", tag="fq"))
                ops.append(('q', alloc))
                for kc in range(KC):
                    for mt in range(2):
                        def mm(kc=kc, mt=mt):
                            nc.tensor.matmul(
                                pq_box[0][:, mt * TCH:(mt + 1) * TCH],
                                wq_sb[:, kc, mt * 128:(mt + 1) * 128],
                                xcs[t][:, kc, :],
                                start=(kc == 0), stop=(kc == KC - 1))
                        ops.append(('q', mm))

                def cp():
                    nc.vector.tensor_copy(
                        out=QTs[t],
                        in_=pq_box[0].rearrange("p (m s) -> p m s", m=2))
                ops.append(('q', cp))
                return ops

            def fc_ops(qc, pool=None, tag="fq"):
                """Feeder items ('fc', op) for fc_out of q-chunk qc."""
                ops = []
                for tt in range(TCH // 128):
                    tok = qc * TCH + tt * 128
                    pf_box = []

                    def alloc(pool=pool, tag=tag):
                        pf_box.append((pool or fqpool).tile(
                            [128, 1024], F32, name=tag, tag=tag))
                    ops.append(('fc', alloc))
                    for hm in range(2):
                        for nch in range(2):
                            def mm(hm=hm, nch=nch, tok=tok):
                                nc.tensor.matmul(
                                    pf_box[0][:, nch * 512:(nch + 1) * 512],
                                    OT2[:, hm, tok:tok + 128],
                                    wo_sb[:, hm, nch * 512:(nch + 1) * 512],
                                    start=(hm == 0), stop=(hm == 1))
                            ops.append(('fc', mm))

                    def fin(tok=tok):
                        ys = ypool.tile([128, 1024], MD, name="ys", tag="ys")
                        nc.vector.tensor_copy(out=ys, in_=pf_box[0])
                        nc.gpsimd.dma_start(out=y[tok:tok + 128, :], in_=ys)
                    ops.append(('fc', fin))
                return ops

            OT2 = otpool.tile([128, 2, SEQ], MD)

            # ---- prologue: first K/V chunk + first Q chunk ----
            kv_proj(0)
            for _, op in q_proj_ops(0):
                op()

            feeders = {
                0: q_proj_ops(1),
                1: q_proj_ops(2) + fc_ops(0),
                2: q_proj_ops(3) + fc_ops(1),
                3: fc_ops(2),
            }

            # ---- fused attention ----
            for qc in range(NT):
                qs = slice(qc * TCH, (qc + 1) * TCH)
                pending = feeders.get(qc, [])
                n_iters_left = 2 * 16
                it = 0

                for hm in range(2):
                    po = popool.tile([D + 1, 2 * TCH], F32, name="po", tag="po")
                    es = [None, None]  # lag-2 pipeline: es[m-2], es[m-1]
                    for m in range(16):
                        # late K/V chunks, right before first use (qc 0 only)
                        if qc == 0 and hm == 0 and m in (4, 8, 12):
                            kv_proj(m // 4)
                        ps = pspool.tile([128, 2 * TCH], F32,
                                         name="ps", tag="ps")
                        for j in range(2):
                            nc.tensor.matmul(
                                ps[:, j * TCH:(j + 1) * TCH],
                                KTs[m // 4][j * D:(j + 1) * D, hm,
                                            (m % 4) * 128:(m % 4 + 1) * 128],
                                QTs[qc][j * D:(j + 1) * D, hm, :],
                                start=True, stop=True)
                        e = epool.tile([128, 2 * TCH], MD, name="e", tag="e")
                        nc.scalar.activation(
                            out=e, in_=ps,
                            func=mybir.ActivationFunctionType.Exp,
                            scale=1.0 / np.sqrt(D))
                        if es[0] is not None:
                            for j in range(2):
                                nc.tensor.matmul(
                                    po[:, j * TCH:(j + 1) * TCH],
                                    Vs[(m - 2) // 4][:, (m - 2) % 4,
                                                     2 * hm + j, :],
                                    es[0][:, j * TCH:(j + 1) * TCH],
                                    start=(m == 2), stop=False)
                        es = [es[1], e]
                        # feed pending work into PE slack (<=2 per iter;
                        # fc gated to iter>=6 of this qc block)
                        n_pop = min(2, -(-len(pending) // n_iters_left)
                                    if n_iters_left > 0 else len(pending))
                        for _ in range(n_pop):
                            if not pending:
                                break
                            tag, op = pending[0]
                            if tag == 'fc' and it < 10:
                                break
                            pending.pop(0)
                            op()
                        n_iters_left -= 1
                        it += 1
                    # drain the two lagged AV pairs
                    for mm_, e_ in ((14, es[0]), (15, es[1])):
                        for j in range(2):
                            nc.tensor.matmul(
                                po[:, j * TCH:(j + 1) * TCH],
                                Vs[3][:, 3 if mm_ == 15 else 2,
                                      2 * hm + j, :],
                                e_[:, j * TCH:(j + 1) * TCH],
                                start=False, stop=(mm_ == 15))

                    # normalize: scratch copy frees psum, then
                    # broadcast(den row) -> reciprocal -> fused mul
                    dn = drpool.tile([1, 2 * TCH], F32, name="dn", tag="dn")
                    nc.vector.tensor_copy(out=dn, in_=po[D:D + 1, :])
                    sc = spool.tile([D + 1, 2 * TCH], F32, name="sc", tag="sc")
                    nc.vector.tensor_copy(out=sc, in_=po)
                    dr = drpool.tile([1, 2 * TCH], F32, name="dr", tag="dr")
                    nc.vector.reciprocal_approx_fast(out=dr, in_=dn)
                    rb = rbpool.tile([D, 2 * TCH], F32, name="rb", tag="rb")
                    nc.gpsimd.partition_broadcast(rb, dr)
                    st = stpool.tile([D, 2 * TCH], MD, name="st", tag="st")
                    nc.vector.tensor_mul(st, sc[0:D, :], rb)
                    for j in range(2):
                        nc.gpsimd.dma_start(
                            out=OT2[j * D:(j + 1) * D, hm, qs],
                            in_=st[:, j * TCH:(j + 1) * TCH])

                while pending:
                    pending.pop(0)[1]()

            # drain fc for the last q-chunk from the (now idle) attention
            # psum pool so consecutive groups pipeline instead of
            # serializing on a single buffer
            for _, op in fc_ops(3, pool=pspool, tag="ps"):
                op()

    nc.compile()
    return nc


def _pack_w(w):
    """(1024, DG) -> (128, KC*DG), row p holding chunks c of rows c*128+p."""
    return np.ascontiguousarray(
        w.reshape(KC, 128, DG).transpose(1, 0, 2).reshape(128, KC * DG))


def shard_inputs(x, Wv, Wk, Wq, Wo):
    import ml_dtypes
    wire = ml_dtypes.bfloat16
    in_maps = []
    for c in range(NCORES):
        n, g = divmod(c, GROUPS)
        cols = slice(g * DG, (g + 1) * DG)
        xt = np.asarray(x[n], np.float32).T  # (EMBED, SEQ)
        xp = xt.reshape(KC, 128, NT, TCH).transpose(2, 1, 0, 3) \
            .reshape(NT, 128, KC * TCH)
        wop = np.asarray(Wo, np.float32)[cols, :] \
            .reshape(DG // 128, 128, EMBED).transpose(1, 0, 2) \
            .reshape(128, (DG // 128) * EMBED)
        in_maps.append({
            "xT": np.ascontiguousarray(xp).astype(wire),
            "wq": _pack_w(np.asarray(Wq, np.float32)[:, cols]).astype(wire),
            "wk": _pack_w(np.asarray(Wk, np.float32)[:, cols]).astype(wire),
            "wv": _pack_w(np.asarray(Wv, np.float32)[:, cols]).astype(wire),
            "wo": np.ascontiguousarray(wop).astype(wire),
        })
    return in_maps


def kernel(x, Wv, Wk, Wq, Wo, bo):
    global LAST_RESULTS
    x = np.asarray(x, np.float32)
    in_maps = shard_inputs(x, Wv, Wk, Wq, Wo)

    if "nc" not in _CACHED_NC:
        _CACHED_NC["nc"] = build_nc()
    nc = _CACHED_NC["nc"]

    trace = os.environ.get("MHA_TRACE", "0") == "1"
    res = bass_utils.run_bass_kernel_spmd(
        nc, in_maps, core_ids=list(range(NCORES)), trace=trace)
    LAST_RESULTS = res

    bo = np.asarray(bo, np.float32)
    out = np.empty((NB, SEQ, EMBED), np.float32)
    for n in range(NB):
        acc = res.results[n * GROUPS]["y"].astype(np.float32)
        for g in range(1, GROUPS):
            acc = acc + res.results[n * GROUPS + g]["y"].astype(np.float32)
        out[n] = acc + bo[None, :]
    return out
